# revision 12
# baseline (speedup 1.0000x reference)
"""Trainium2 Bass kernel for nn_CIFARnet (8-step NOSO spiking CNN).

- Pure data parallel: 8 cores, one batch element each; weights replicated.
- Layer-major over time: conv of spikes once per layer (um(t)=DM*um(t-1)+conv(sp(t))
  since conv is linear); leaky integration via tensor_tensor_scan (t-minor).
- fp32r matmuls (spike inputs exact); elementwise fp32 matches reference.
- Event-driven skip: flat chain of runtime guards on per-layer spike counts;
  an all-zero layer is skipped exactly.
- All SBUF/PSUM buffers are persistent depth-0 allocations (no pool slot
  handoff across conditional blocks, which deadlocks the tile scheduler).
- sum_sp partial per core, summed on host (== final psum).
"""
import numpy as np

import concourse.bass as bass
import concourse.bacc as bacc
import concourse.mybir as mybir
import concourse.tile as tile
from concourse.bass_utils import run_bass_kernel_spmd

f32 = mybir.dt.float32
f32r = mybir.dt.float32r
bf16 = mybir.dt.bfloat16
i32 = mybir.dt.int32
u32 = mybir.dt.uint32
AT = mybir.AluOpType
AX = mybir.AxisListType
AF = mybir.ActivationFunctionType

DM = float(np.float32(np.exp(-1.0 / 20.0)))
DS = float(np.float32(np.exp(-1.0 / 5.0)))
SCALE = 0.2
T = 8
NCORES = 8
PATF = 512

_nc_cache = {}


def build_nc():
    nc = bacc.Bacc("TRN2", target_bir_lowering=False, debug=False,
                   num_devices=NCORES)

    img_d = nc.dram_tensor("img", [3, 32, 32], f32, kind="ExternalInput")
    w1_d = nc.dram_tensor("w1p", [16, 5, 96, 128], f32, kind="ExternalInput")
    w2_d = nc.dram_tensor("w2p", [25, 64, 128], f32, kind="ExternalInput")
    w3_d = nc.dram_tensor("w3p", [50, 128, 128], f32, kind="ExternalInput")
    w4_d = nc.dram_tensor("w4p", [4, 50, 128, 128], f32, kind="ExternalInput")
    w5_d = nc.dram_tensor("w5p", [2, 100, 128, 128], f32, kind="ExternalInput")
    wf1_d = nc.dram_tensor("wf1p", [128, 8, 128, 128], f32,
                           kind="ExternalInput")
    wf2_d = nc.dram_tensor("wf2p", [32, 128, 128], f32, kind="ExternalInput")
    wf3_d = nc.dram_tensor("wf3p", [4, 128, 10], f32, kind="ExternalInput")

    outt_d = nc.dram_tensor("out_t", [10], f32, kind="ExternalOutput")
    outu_d = nc.dram_tensor("out_u", [10], f32, kind="ExternalOutput")
    ssp_d = nc.dram_tensor("ssp", [9], f32, kind="ExternalOutput")

    sp5_d = nc.dram_tensor("sp5d", [16384, 8], f32)

    with tile.TileContext(nc) as tc:
        with (
            tc.tile_pool(name="per", bufs=1) as per,
            tc.tile_pool(name="pp", bufs=1, space="PSUM") as pp,
        ):
            # ---------- persistent buffers ----------
            patDM = per.tile([128, PATF], f32)
            patDS = per.tile([128, PATF], f32)
            pat1f = per.tile([128, PATF], f32)
            pat1b = per.tile([128, PATF], bf16)
            ssp_sb = per.tile([1, 16], f32)

            img96 = per.tile([96, 32], f32)
            sx = per.tile([96, 32], f32)
            uenc = per.tile([96, 32], f32)
            keep = per.tile([96, 32], f32)
            spenc = per.tile([96, 36 * T], f32r)
            encr = per.tile([96, 1], f32)

            c1 = per.tile([128, 4096], f32)
            sp1 = per.tile([128, 4096], f32)

            um = per.tile([128, PATF], f32)
            us = per.tile([128, PATF], f32)
            dd = per.tile([128, PATF], f32)
            ab = per.tile([128, PATF], bf16)
            cmt = per.tile([128, PATF], bf16)
            nz4 = per.tile([128, PATF], bf16)
            nzc = per.tile([128, PATF], bf16)
            p1e = per.tile([128, PATF], bf16)
            r4t = per.tile([128, PATF], bf16)
            lrt = per.tile([128, PATF], bf16)
            sph = per.tile([128, PATF], bf16)
            spb = per.tile([128, PATF], bf16)
            vq0 = per.tile([128, PATF // 4], bf16)
            vq1 = per.tile([128, PATF // 4], bf16)
            vq2 = per.tile([128, PATF // 4], bf16)
            vq3 = per.tile([128, PATF // 4], bf16)
            vqs = [vq0, vq1, vq2, vq3]

            wA = per.tile([128, 1664], f32r)
            wB = per.tile([128, 1664], f32r)
            wAB = [wA, wB]

            in2 = per.tile([64, 2 * 36 * 36], f32r)     # 2 t-planes
            c2h = per.tile([128, 4096], f32)            # half (yh) of c2
            in3 = per.tile([128, T * 400], f32r)
            cnt2 = per.tile([128, 16], f32)
            c3h = per.tile([128, 2048], f32)            # per-cc half of c3
            in4 = per.tile([128, 2 * T * 144], f32r)
            cnt3 = per.tile([128, 8], f32)
            c4 = per.tile([128, 2048], f32)
            cum4 = per.tile([128, 2048], bf16)
            in5 = per.tile([128, 4 * T * 144], f32r)
            c5 = per.tile([128, 1024], f32)
            cum5 = per.tile([128, 1024], bf16)
            sp5t = per.tile([128, 1024], f32)
            sp5sb = per.tile([128, 1024], f32r)
            cf1 = per.tile([128, 64], f32)
            spf1 = per.tile([128, 64], f32r)
            cumf1 = per.tile([128, 64], bf16)
            cf2 = per.tile([128, 32], f32)
            spf2 = per.tile([128, 32], f32r)
            cumf2 = per.tile([128, 32], bf16)
            cf3 = per.tile([10, 8], f32)
            um3 = per.tile([10, 8], f32)
            us3 = per.tile([10, 8], f32)
            ab3 = per.tile([10, 8], bf16)
            d3 = per.tile([10, 8], f32)
            spf3 = per.tile([10, 8], f32)
            cum3 = per.tile([10, 8], bf16)
            r1 = per.tile([128, 1], f32)
            rX = per.tile([128, 1], f32)
            rf3 = per.tile([10, 1], f32)
            nzd = per.tile([10, 8], f32)
            savc = per.tile([10, 8], f32)
            nz3 = per.tile([10, 8], f32)
            nzc3 = per.tile([10, 8], f32)
            tes = per.tile([10, 8], f32)
            q8 = per.tile([10, 8], f32)
            ou8 = per.tile([10, 8], f32)
            outt = per.tile([10, 1], f32)
            outu = per.tile([10, 1], f32)

            pp0 = pp.tile([128, 512], f32, tag="pp0")
            pp1 = pp.tile([128, 512], f32, tag="pp1")
            ppF = pp.tile([128, 64], f32, tag="ppF")
            ppH = pp.tile([10, 8], f32, tag="ppH")

            # ---------- patterns ----------
            for (pt_, v) in ((patDM, DM), (patDS, DS), (pat1f, 1.0),
                             (pat1b, 1.0)):
                nc.vector.memset(pt_[:], v)
                pv = pt_[:].rearrange("p (x t) -> p x t", t=T)
                nc.vector.memset(pv[:, :, 0:1], 0.0)

            def scan(out, d0, d1, op0, op1):
                nc.vector.tensor_tensor_scan(out, d0, d1, 0.0, op0, op1)

            # ---------- encoder ----------
            nc.sync.dma_start(img96[:], img_d[:].transpose([1, 0, 2]))
            nc.vector.tensor_scalar(sx[:], img96[:], SCALE, None, AT.mult)
            nc.vector.memset(uenc[:], 0.0)
            nc.vector.memset(spenc[:].bitcast(u32), 0)
            spev = spenc[:].rearrange("p (x t) -> p x t", t=T)
            for t in range(T):
                nc.vector.tensor_scalar(uenc[:], uenc[:], DM, None, AT.mult)
                nc.vector.tensor_tensor(uenc[:], uenc[:], sx[:], AT.add)
                nc.vector.tensor_scalar(spev[:, 2:34, t], uenc[:], 1.0, None,
                                        AT.is_gt)
                nc.vector.tensor_scalar(keep[:], uenc[:], 1.0, None, AT.is_le)
                nc.vector.tensor_tensor(uenc[:], uenc[:], keep[:], AT.mult)
            nc.vector.tensor_reduce(encr[:], spev[:, 2:34, :].bitcast(f32),
                                    AX.XY, AT.add)
            nc.gpsimd.tensor_reduce(ssp_sb[0:1, 0:1], encr[:], AX.C, AT.add)

            # ---------- L1 (always on) ----------
            for q in range(16):
                w1sb = wAB[q % 2]
                nc.sync.dma_start(
                    w1sb[0:96, 0:640],
                    w1_d[q].transpose([1, 0, 2]).bitcast(f32r))
                pt = pp0 if (q % 2 == 0) else pp1
                for dx in range(5):
                    nc.tensor.matmul(pt[:, 0:256],
                                     w1sb[0:96, dx * 128:(dx + 1) * 128],
                                     spev[:, dx:dx + 32, :],
                                     start=(dx == 0), stop=(dx == 4))
                c1v = c1[:].rearrange("p (q x t) -> p q x t", x=32, t=T)
                nc.scalar.activation(
                    c1v[:, q],
                    pt[:, 0:256].rearrange("p (x t) -> p x t", t=T), AF.Copy)

            def scan_chunk(cv, sz, spike_to=None, cum_to=None):
                scan(um[:, :sz], patDM[:, :sz], cv, AT.mult, AT.add)
                scan(us[:, :sz], patDS[:, :sz], cv, AT.mult, AT.add)
                nc.vector.tensor_tensor(dd[:, :sz], um[:, :sz], us[:, :sz],
                                        AT.subtract)
                nc.vector.tensor_scalar(ab[:, :sz], dd[:, :sz], 1.0, None,
                                        AT.is_gt)
                cumv = cmt[:, :sz] if cum_to is None else cum_to
                scan(cumv, pat1b[:, :sz], ab[:, :sz], AT.mult, AT.max)
                cuv = cumv.rearrange("p (x t) -> p x t", t=T)
                if spike_to is not None:
                    nc.vector.tensor_tensor(spike_to[:, :, 1:8],
                                            cuv[:, :, 1:8], cuv[:, :, 0:7],
                                            AT.subtract)
                    nc.vector.tensor_copy(spike_to[:, :, 0:1],
                                          cuv[:, :, 0:1])
                return cuv

            sp1tm = sp1[:].rearrange("p (t x) -> p x t", x=512)
            for h in range(8):
                cv = c1[:, h * 512:(h + 1) * 512]
                spv = sp1tm[:, h * 64:(h + 1) * 64, :]
                scan_chunk(cv, 512, spike_to=spv)
            nc.vector.tensor_reduce(r1[:], sp1[:], AX.X, AT.add)
            nc.gpsimd.tensor_reduce(ssp_sb[0:1, 1:2], r1[:], AX.C, AT.add)

            def fill_zeros(from_slot):
                nc.vector.memset(ssp_sb[0:1, from_slot:9], 0.0)
                nc.vector.memset(d3[:], 0.0)
                nc.vector.memset(spf3[:], 0.0)
                nc.vector.memset(cum3[:], 0.0)

            def load_guard(slot):
                g = nc.alloc_registers()
                nc.regs_load(g, ssp_sb[0:1, slot:slot + 1].bitcast(i32))
                return g

            def pool_chunk(cv_c, sz, cnt_col, W, emit):
                # nz4 = 4*(c != 0) via square + is_gt (mod/ne not in ISA)
                nc.scalar.square(dd[:, :sz], cv_c)
                nc.vector.tensor_scalar(nz4[:, :sz], dd[:, :sz], 0.0, 4.0,
                                        AT.is_gt, AT.mult)
                spbv = spb[:, :sz].rearrange("p (x t) -> p x t", t=T)
                scan_chunk(cv_c, sz, spike_to=spbv)
                nc.vector.tensor_reduce(cnt_col, spb[:, :sz], AX.X, AT.add)
                scan(nzc[:, :sz], pat1b[:, :sz], nz4[:, :sz], AT.mult, AT.max)
                scan(p1e[:, :sz], pat1b[:, :sz], nzc[:, :sz], AT.mult, AT.add)
                nc.vector.tensor_scalar(p1e[:, :sz], p1e[:, :sz], 28.0, None,
                                        AT.subtract)
                nc.vector.tensor_tensor(r4t[:, :sz], spb[:, :sz], p1e[:, :sz],
                                        AT.mult)
                scan(lrt[:, :sz], pat1b[:, :sz], r4t[:, :sz], AT.mult, AT.add)
                # A = lr - 0.5*spb (into sph); B = lr
                nc.vector.tensor_scalar(sph[:, :sz], spb[:, :sz], 0.5, None,
                                        AT.mult)
                nc.vector.tensor_tensor(sph[:, :sz], lrt[:, :sz], sph[:, :sz],
                                        AT.subtract)
                av = sph[:, :sz].rearrange("p (y x t) -> p y x t", x=W, t=T)
                bv = lrt[:, :sz].rearrange("p (y x t) -> p y x t", x=W, t=T)
                q4 = sz // 4
                for k in range(4):
                    dy, dx = k // 2, k % 2
                    qv = vqs[k][:, :q4].rearrange("p (y x t) -> p y x t",
                                                  x=W // 2, t=T)
                    nc.vector.tensor_scalar(qv, av[:, dy::2, dx::2, :],
                                            32.0 + k, None, AT.add)
                nc.vector.tensor_tensor(vq0[:, :q4], vq0[:, :q4],
                                        vq1[:, :q4], AT.min)
                nc.vector.tensor_tensor(vq2[:, :q4], vq2[:, :q4],
                                        vq3[:, :q4], AT.min)
                nc.vector.tensor_tensor(vq0[:, :q4], vq0[:, :q4],
                                        vq2[:, :q4], AT.min)  # Amin
                for k in range(4):
                    dy, dx = k // 2, k % 2
                    qv = vqs[1 if k < 2 else (2 if k == 2 else 3)]
                    qvv = qv[:, :q4].rearrange("p (y x t) -> p y x t",
                                               x=W // 2, t=T)
                    if k == 1:
                        tmpq = vq2
                        qvv = tmpq[:, :q4].rearrange(
                            "p (y x t) -> p y x t", x=W // 2, t=T)
                    nc.vector.tensor_scalar(qvv, bv[:, dy::2, dx::2, :],
                                            32.0 + k, None, AT.add)
                    if k == 1:
                        nc.vector.tensor_tensor(vq1[:, :q4], vq1[:, :q4],
                                                vq2[:, :q4], AT.min)
                nc.vector.tensor_tensor(vq2[:, :q4], vq2[:, :q4],
                                        vq3[:, :q4], AT.min)
                nc.vector.tensor_tensor(vq1[:, :q4], vq1[:, :q4],
                                        vq2[:, :q4], AT.min)  # Bmin
                nc.vector.tensor_tensor(vq1[:, :q4], vq1[:, :q4],
                                        vq0[:, :q4], AT.subtract)  # 0.5*s
                mq = vq1[:, :q4].rearrange("p (y x t) -> p y x t",
                                           x=W // 2, t=T)
                emit(mq)

            # ================= L2 =================
            in2v = in2[:].rearrange("p (t y x) -> p t y x", y=36, x=36)
            in3v = in3[:].rearrange("p (t y x) -> p t y x", y=20, x=20)
            g1 = load_guard(1)
            with tc.If(nc.snap(g1) == 0) as if1:
                fill_zeros(2)
            with if1.Else():
                nc.vector.memset(in3[:].bitcast(u32), 0)
                for yh in range(2):
                    for th in range(4):
                        nc.vector.memset(in2[:].bitcast(u32), 0)
                        for j in range(2):
                            for tl in range(2):
                                s = sp1[j * 64:(j + 1) * 64, :].rearrange(
                                    "p (t q x) -> p t q x", q=16, x=32
                                )[:, th * 2 + tl]
                                dst = in2v[:, tl, 2 + j:34 + j:2, 2:34]
                                nc.sync.dma_start(dst, s.bitcast(f32r))
                        for tl in range(2):
                            t = th * 2 + tl
                            pt = pp0 if (tl == 0) else pp1
                            for kc in range(2):
                                wsb = wAB[kc]
                                ks, ke = (0, 13) if kc == 0 else (13, 25)
                                nc.sync.dma_start(
                                    wsb[0:64, 0:(ke - ks) * 128],
                                    w2_d[ks:ke].transpose(
                                        [1, 0, 2]).bitcast(f32r))
                                for k in range(ks, ke):
                                    dy, dx = k // 5, k % 5
                                    nc.tensor.matmul(
                                        pt[:],
                                        wsb[0:64, (k - ks) * 128:
                                            (k - ks + 1) * 128],
                                        in2v[:, tl,
                                             yh * 16 + dy:yh * 16 + dy + 16,
                                             dx:dx + 32],
                                        start=(k == 0), stop=(k == 24))
                            c2v = c2h[:].rearrange("p (y x t) -> p y x t",
                                                   x=32, t=T)
                            nc.scalar.activation(
                                c2v[:, :, :, t],
                                pt[:].rearrange("p (y x) -> p y x", x=32),
                                AF.Copy)
                    for h in range(8):
                        grow = yh * 8 + h

                        def emit2(mq, grow=grow):
                            dst = in3v[:, :, 2 + grow:3 + grow,
                                       2:18].transpose([0, 2, 3, 1])
                            nc.vector.tensor_scalar(dst, mq, 2.0, None,
                                                    AT.mult)

                        pool_chunk(c2h[:, h * 512:(h + 1) * 512], 512,
                                   cnt2[:, grow:grow + 1], 32, emit2)
                nc.vector.tensor_reduce(rX[:], cnt2[:], AX.X, AT.add)
                nc.gpsimd.tensor_reduce(ssp_sb[0:1, 2:3], rX[:], AX.C, AT.add)

            # ================= L3 =================
            in4v = in4[:].rearrange("p (c t y x) -> p c t y x",
                                    c=2, y=12, x=12)
            g2 = load_guard(2)
            with tc.If(nc.snap(g2) == 0) as if2:
                fill_zeros(3)
            with if2.Else():
                nc.vector.memset(in4[:].bitcast(u32), 0)
                for cc in range(2):
                    for t in range(T):
                        pt = pp0 if (t % 2 == 0) else pp1
                        for kc in range(2):
                            wsb = wAB[kc]
                            ks, ke = (0, 13) if kc == 0 else (13, 25)
                            nc.sync.dma_start(
                                wsb[:, 0:(ke - ks) * 128],
                                w3_d[ks * 2 + cc:ke * 2:2].transpose(
                                    [1, 0, 2]).bitcast(f32r))
                            for k in range(ks, ke):
                                dy, dx = k // 5, k % 5
                                nc.tensor.matmul(
                                    pt[:, 0:256],
                                    wsb[:, (k - ks) * 128:(k - ks + 1) * 128],
                                    in3v[:, t, dy:dy + 16, dx:dx + 16],
                                    start=(k == 0), stop=(k == 24))
                        c3v = c3h[:].rearrange("p (y x t) -> p y x t",
                                               x=16, t=T)
                        nc.scalar.activation(
                            c3v[:, :, :, t],
                            pt[:, 0:256].rearrange("p (y x) -> p y x", x=16),
                            AF.Copy)
                    for h in range(4):
                        def emit3(mq, cc=cc, h=h):
                            dst = in4v[:, cc, :, 2 + h * 2:4 + h * 2,
                                       2:10].transpose([0, 2, 3, 1])
                            nc.vector.tensor_scalar(dst, mq, 2.0, None,
                                                    AT.mult)

                        pool_chunk(c3h[:, h * 512:(h + 1) * 512], 512,
                                   cnt3[:, cc * 4 + h:cc * 4 + h + 1], 16,
                                   emit3)
                nc.vector.tensor_reduce(rX[:], cnt3[:], AX.X, AT.add)
                nc.gpsimd.tensor_reduce(ssp_sb[0:1, 3:4], rX[:], AX.C, AT.add)

            # ================= L4 =================
            in5v = in5[:].rearrange("p (c t y x) -> p c t y x",
                                    c=4, y=12, x=12)
            g3 = load_guard(3)
            with tc.If(nc.snap(g3) == 0) as if3:
                fill_zeros(4)
            with if3.Else():
                nc.vector.memset(in5[:].bitcast(u32), 0)
                for oc in range(4):
                    for tq in range(2):
                        pt = pp0 if (tq == 0) else pp1
                        n = 0
                        for kc in range(5):
                            wsb = wAB[kc % 2]
                            ks = (0, 5, 10, 15, 20)[kc]
                            ke = (5, 10, 15, 20, 25)[kc]
                            nc.sync.dma_start(
                                wsb[:, 0:(ke - ks) * 2 * 128],
                                w4_d[oc][ks * 2:ke * 2].transpose(
                                    [1, 0, 2]).bitcast(f32r))
                            for k in range(ks, ke):
                                dy, dx = k // 5, k % 5
                                for ic in range(2):
                                    nc.tensor.matmul(
                                        pt[:, 0:256],
                                        wsb[:, ((k - ks) * 2 + ic) * 128:
                                            ((k - ks) * 2 + ic + 1) * 128],
                                        in4v[:, ic, tq * 4:tq * 4 + 4,
                                             dy:dy + 8, dx:dx + 8],
                                        start=(n == 0), stop=(n == 49))
                                    n += 1
                        c4v = c4[:].rearrange("p (oc y x t) -> p oc y x t",
                                              oc=4, y=8, x=8)
                        nc.scalar.activation(
                            c4v[:, oc, :, :, tq * 4:tq * 4 + 4]
                            .transpose([0, 3, 1, 2]),
                            pt[:, 0:256].rearrange("p (t y x) -> p t y x",
                                                   y=8, x=8), AF.Copy)
                cum4r = cum4[:].rearrange("p (oc y x t) -> p oc y x t",
                                          oc=4, y=8, x=8)
                for h in range(4):
                    scan_chunk(c4[:, h * 512:(h + 1) * 512], 512,
                               cum_to=cum4[:, h * 512:(h + 1) * 512])
                    oc = h
                    dst = in5v[:, oc, :, 2:10, 2:10].transpose([0, 2, 3, 1])
                    cv_ = cum4r[:, oc]
                    nc.vector.tensor_tensor(dst[:, :, :, 1:8],
                                            cv_[:, :, :, 1:8],
                                            cv_[:, :, :, 0:7], AT.subtract)
                    nc.vector.tensor_copy(dst[:, :, :, 0:1],
                                          cv_[:, :, :, 0:1])
                c4l = cum4[:].rearrange("p (x t) -> p x t", t=T)
                nc.vector.tensor_reduce(rX[:], c4l[:, :, 7:8], AX.XY, AT.add)
                nc.gpsimd.tensor_reduce(ssp_sb[0:1, 4:5], rX[:], AX.C, AT.add)

            # ================= L5 =================
            g4 = load_guard(4)
            with tc.If(nc.snap(g4) == 0) as if4:
                fill_zeros(5)
            with if4.Else():
                B5 = [0, 3, 6, 9, 12, 15, 18, 21, 24, 25]
                for oc in range(2):
                    for kc in range(9):
                        wsb = wAB[kc % 2]
                        ks = B5[kc]
                        ke = B5[kc + 1]
                        nc.sync.dma_start(
                            wsb[:, 0:(ke - ks) * 4 * 128],
                            w5_d[oc][ks * 4:ke * 4].transpose(
                                [1, 0, 2]).bitcast(f32r))
                        for tq in range(2):
                            pt = pp0 if (tq == 0) else pp1
                            m = 0
                            for k in range(ks, ke):
                                dy, dx = k // 5, k % 5
                                for ic in range(4):
                                    nc.tensor.matmul(
                                        pt[:, 0:256],
                                        wsb[:, ((k - ks) * 4 + ic) * 128:
                                            ((k - ks) * 4 + ic + 1) * 128],
                                        in5v[:, ic, tq * 4:tq * 4 + 4,
                                             dy:dy + 8, dx:dx + 8],
                                        start=(kc == 0 and m == 0),
                                        stop=(kc == 8 and
                                              m == (ke - ks) * 4 - 1),
                                        skip_group_check=True)
                                    m += 1
                    for tq in range(2):
                        pt = pp0 if (tq == 0) else pp1
                        c5v = c5[:].rearrange("p (oc y x t) -> p oc y x t",
                                              oc=2, y=8, x=8)
                        nc.scalar.activation(
                            c5v[:, oc, :, :, tq * 4:tq * 4 + 4]
                            .transpose([0, 3, 1, 2]),
                            pt[:, 0:256].rearrange("p (t y x) -> p t y x",
                                                   y=8, x=8), AF.Copy)
                spv5 = sp5t[:].rearrange("p (x t) -> p x t", t=T)
                for h in range(2):
                    scan_chunk(c5[:, h * 512:(h + 1) * 512], 512,
                               spike_to=spv5[:, h * 64:(h + 1) * 64, :],
                               cum_to=cum5[:, h * 512:(h + 1) * 512])
                c5l = cum5[:].rearrange("p (x t) -> p x t", t=T)
                nc.vector.tensor_reduce(rX[:], c5l[:, :, 7:8], AX.XY, AT.add)
                nc.gpsimd.tensor_reduce(ssp_sb[0:1, 5:6], rX[:], AX.C, AT.add)

            # ================= f1 =================
            g5 = load_guard(5)
            with tc.If(nc.snap(g5) == 0) as if5:
                fill_zeros(6)
            with if5.Else():
                dstv = sp5_d[:].rearrange("(oc p x) t -> p oc x t",
                                          oc=2, p=128)
                srcv = sp5t[:].rearrange("p (oc x t) -> p oc x t", oc=2, t=T)
                for oc in range(2):
                    nc.sync.dma_start(dstv[:, oc], srcv[:, oc])
                nc.sync.dma_start(
                    sp5sb[:].rearrange("p (c t) -> p c t", t=T),
                    sp5_d[:].rearrange("(c p) t -> p c t",
                                       p=128).bitcast(f32r))
                for c in range(128):
                    wt = wAB[c % 2]
                    nc.sync.dma_start(
                        wt[:, 0:1024],
                        wf1_d[c].transpose([1, 0, 2]).bitcast(f32r))
                    for o in range(8):
                        nc.tensor.matmul(
                            ppF[:, o * 8:(o + 1) * 8],
                            wt[:, o * 128:(o + 1) * 128],
                            sp5sb[:, c * 8:(c + 1) * 8],
                            start=(c == 0), stop=(c == 127),
                            skip_group_check=True)
                nc.scalar.activation(cf1[:], ppF[:], AF.Copy)
                spf1v = spf1[:].rearrange("p (x t) -> p x t", t=T)
                scan_chunk(cf1[:, :64], 64, spike_to=spf1v,
                           cum_to=cumf1[:, :64])
                cf1l = cumf1[:].rearrange("p (x t) -> p x t", t=T)
                nc.vector.tensor_reduce(rX[:], cf1l[:, :, 7:8], AX.XY, AT.add)
                nc.gpsimd.tensor_reduce(ssp_sb[0:1, 6:7], rX[:], AX.C, AT.add)

            # ================= f2 =================
            g6 = load_guard(6)
            with tc.If(nc.snap(g6) == 0) as if6:
                fill_zeros(7)
            with if6.Else():
                for ic in range(8):
                    wt = wAB[ic % 2]
                    nc.sync.dma_start(
                        wt[:, 0:512],
                        wf2_d[ic * 4:(ic + 1) * 4].transpose(
                            [1, 0, 2]).bitcast(f32r))
                    for o in range(4):
                        nc.tensor.matmul(
                            ppF[:, o * 8:(o + 1) * 8],
                            wt[:, o * 128:(o + 1) * 128],
                            spf1[:, ic * 8:(ic + 1) * 8],
                            start=(ic == 0), stop=(ic == 7),
                            skip_group_check=True)
                nc.scalar.activation(cf2[:], ppF[:, 0:32], AF.Copy)
                spf2v = spf2[:].rearrange("p (x t) -> p x t", t=T)
                scan_chunk(cf2[:, :32], 32, spike_to=spf2v,
                           cum_to=cumf2[:, :32])
                cf2l = cumf2[:].rearrange("p (x t) -> p x t", t=T)
                nc.vector.tensor_reduce(rX[:], cf2l[:, :, 7:8], AX.XY, AT.add)
                nc.gpsimd.tensor_reduce(ssp_sb[0:1, 7:8], rX[:], AX.C, AT.add)

            # ================= f3 =================
            g7 = load_guard(7)
            with tc.If(nc.snap(g7) == 0) as if7:
                fill_zeros(8)
            with if7.Else():
                nc.sync.dma_start(
                    wA[:, 0:40],
                    wf3_d[:].transpose([1, 0, 2]).bitcast(f32r))
                for ic in range(4):
                    nc.tensor.matmul(
                        ppH[:],
                        wA[:, ic * 10:ic * 10 + 10],
                        spf2[:, ic * 8:(ic + 1) * 8],
                        start=(ic == 0), stop=(ic == 3))
                nc.scalar.activation(cf3[:], ppH[:], AF.Copy)
                scan(um3[:], patDM[0:10, 0:8], cf3[:], AT.mult, AT.add)
                scan(us3[:], patDS[0:10, 0:8], cf3[:], AT.mult, AT.add)
                nc.vector.tensor_tensor(d3[:], um3[:], us3[:], AT.subtract)
                nc.vector.tensor_scalar(ab3[:], d3[:], 1.0, None, AT.is_gt)
                scan(cum3[:], pat1b[0:10, 0:8], ab3[:], AT.mult, AT.max)
                nc.vector.tensor_tensor(spf3[:, 1:8], cum3[:, 1:8],
                                        cum3[:, 0:7], AT.subtract)
                nc.vector.tensor_copy(spf3[:, 0:1], cum3[:, 0:1])
                nc.vector.tensor_reduce(rf3[:], spf3[:], AX.X, AT.add)
                nc.gpsimd.tensor_reduce(ssp_sb[0:1, 8:9], rf3[:], AX.C,
                                        AT.add)

            # ---------- always-on tail ----------
            nc.scalar.square(q8[:], d3[:])
            nc.vector.tensor_scalar(nzd[:], q8[:], 0.0, None, AT.is_gt)
            nc.vector.memset(savc[:, 0:1], 1.0)
            nc.vector.tensor_scalar(savc[:, 1:8], cum3[:, 0:7], -1.0, 1.0,
                                    AT.mult, AT.add)
            nc.vector.tensor_tensor(nz3[:], nzd[:], savc[:], AT.mult)
            scan(nzc3[:], pat1f[0:10, 0:8], nz3[:], AT.mult, AT.max)
            scan(tes[:], pat1f[0:10, 0:8], nzc3[:], AT.mult, AT.add)
            nc.vector.tensor_scalar(q8[:], tes[:], 8.0, None, AT.subtract)
            nc.vector.tensor_tensor(q8[:], spf3[:], q8[:], AT.mult)
            nc.vector.tensor_reduce(outt[:], q8[:], AX.X, AT.add)
            nc.vector.tensor_scalar(outt[:], outt[:], 8.0, None, AT.add)
            nc.vector.tensor_tensor(ou8[:], spf3[:], d3[:], AT.mult)
            nc.vector.tensor_reduce(outu[:], ou8[:], AX.X, AT.add)

            nc.sync.dma_start(outt_d[:].unsqueeze(1), outt[:])
            nc.sync.dma_start(outu_d[:].unsqueeze(1), outu[:])
            nc.sync.dma_start(ssp_d[:].unsqueeze(0), ssp_sb[0:1, 0:9])

    nc.finalize()
    return nc


# ====================== host-side packing ======================

def _pack_weights(wc1, wc2, wc3, wc4, wc5, wf1, wf2, wf3):
    w1p = np.zeros((16, 5, 96, 128), np.float32)
    for qq in range(16):
        for j in range(2):
            for dy in range(5):
                yin = 2 * qq + j + dy - 2
                if 0 <= yin < 32:
                    arr = wc1[:, :, dy, :].transpose(2, 1, 0)  # [dx, c, co]
                    w1p[qq, :, yin * 3:yin * 3 + 3, j * 64:(j + 1) * 64] = arr
    w2p = np.ascontiguousarray(wc2.transpose(2, 3, 1, 0).reshape(25, 64, 128))
    w3p = np.ascontiguousarray(
        wc3.transpose(2, 3, 1, 0).reshape(25, 128, 2, 128)
        .transpose(0, 2, 1, 3).reshape(50, 128, 128))
    w4p = np.ascontiguousarray(
        wc4.transpose(2, 3, 1, 0).reshape(25, 2, 128, 4, 128)
        .transpose(3, 0, 1, 2, 4).reshape(4, 50, 128, 128))
    w5p = np.ascontiguousarray(
        wc5.transpose(2, 3, 1, 0).reshape(25, 4, 128, 2, 128)
        .transpose(3, 0, 1, 2, 4).reshape(2, 100, 128, 128))
    wf1p = np.ascontiguousarray(
        wf1.T.reshape(128, 128, 8, 128).transpose(0, 2, 1, 3))
    wf2p = np.ascontiguousarray(
        wf2.T.reshape(8, 128, 4, 128).transpose(0, 2, 1, 3)
        .reshape(32, 128, 128))
    wf3p = np.ascontiguousarray(wf3.T.reshape(4, 128, 10))
    return dict(w1p=w1p, w2p=w2p, w3p=w3p, w4p=w4p, w5p=w5p,
                wf1p=wf1p, wf2p=wf2p, wf3p=wf3p)


def kernel(input, wc1, wc2, wc3, wc4, wc5, wf1, wf2, wf3, batch_size):
    input = np.asarray(input, np.float32)
    packed = _pack_weights(*[np.asarray(w, np.float32) for w in
                             (wc1, wc2, wc3, wc4, wc5, wf1, wf2, wf3)])
    if "nc" not in _nc_cache:
        _nc_cache["nc"] = build_nc()
    nc = _nc_cache["nc"]
    in_maps = []
    for b in range(NCORES):
        m = {"img": np.ascontiguousarray(input[b])}
        m.update(packed)
        in_maps.append(m)
    res = run_bass_kernel_spmd(nc, in_maps, list(range(NCORES)))
    out_t = np.stack([res.results[b]["out_t"] for b in range(NCORES)])
    out_u = np.stack([res.results[b]["out_u"] for b in range(NCORES)])
    sum_sp = np.sum([res.results[b]["ssp"] for b in range(NCORES)], axis=0)
    return out_t, out_u, sum_sp.astype(np.float32)
